# revision 21
# baseline (speedup 1.0000x reference)
"""Trainium2 Bass kernel for: softmax2d(channel) -> channel mix -> bias ->
RReLU(0.2 eval) -> relu(mixed + 0.1*x).

Full-input contract: kernel(**inputs) takes the complete tensors and returns
the complete output. Internally shards batch B=16 across 8 NeuronCores
(2 batches/core). Per-core layout: [128 partitions = 2 batches x 64 channels,
65536 free = H*W].

v5: fp16 I/O (DMA 187->93 us/core) + fused recip-multiply custom DVE op.

Host uploads xs = f16(0.1*x); ACT exp uses scale=10 so e = exp(x).
With W'[(b,c),(b,d)] = mix[d,c] + bias[d] (bias folded via sum_c e = S):
  V' = W' @ e = S*(mix@softmax + bias),  SB = blockones @ e = S (bcast)
  aa = prelu(V')   ACT Prelu (psum->f16; shares exp's act table) with a few
                   chunks on a custom DVE op maxx(0.2*v, v) for balance
  t  = aa * 1/SB   ONE custom DVE op (bit-trick seed + 1 Newton + multiply,
                   ~0.4% rel err) - replaces recip + full multiply pass
  z  = relu(xs+t)  tt-add f16 (2x) + ts-max f16 (4x); even chunks take both
                   steps on Pool (same-chunk pairing schedules best)

ISA constraints found the hard way: no divide op on any engine; at most one
PSUM operand per DVE/Pool instruction (even the same AP twice); Pool reads
SBUF only and only tt/ts classes (TensorScalarPtr scan variant is rejected
by walrus codegen on Pool); custom/stt DVE ops get no 2x/4x modes; matmul
outputs f32 PSUM only; DMA cannot read PSUM (so S can never be compacted
out of PSUM cheaply - engines charge per free-column regardless of
partition count); AF.Reciprocal blocked, exp/reciprocal tables thrash.
Small head/tail tiles trim pipeline fill and drain.

v6: per-chunk engine routing tables (ADD_ROUTE/RELU_ROUTE/PRELU_DVE_SET)
hill-climbed against TimelineSim. The v5 even/odd same-chunk pairing is a
sharp scheduler local optimum: the tile list-scheduler costs Pool with v1
(no gpsimd efficiency penalty, 2.4x optimistic for tt-add), so any large
rebalance (wider Pool chunks, scan-adds, all-relu-on-DVE, gated manual
schedules, 2048 PSUM chunks) lowers busy but adds 5-70us of queue-convoy
stalls. Only single-chunk boundary flips survived: add1->pool,
add52->dve, relu62->pool, relu46->dve, prelu56->ACT. Engine busy
(TimelineSim): DVE 118.6, ACT 115.6, Pool 113.4, DMA 93.6, PE 56.6 ->
exec 133.46 us (vs 134.45 v5, 196.6 original).
"""

import numpy as np

B, C, H, W = 16, 64, 256, 256
N_CORES = 8
BPC = B // N_CORES          # batches per core
P = BPC * C                 # 128 partitions
F = H * W                   # 65536 free columns per core
TILE_N = 4096               # SBUF tile width
PS_V_N = 1024               # V' PSUM chunk width (prelu granularity)
PS_S_N = 1024               # S PSUM chunk width (recip_mul granularity)
MM_N = 512                 # single matmul free dim (1 PSUM bank)
RRELU_SLOPE = 0.2
X_BUFS = 5
E_BUFS = 3
AA_BUFS = 3
T_BUFS = 3
Z_BUFS = 4
PS_V_BUFS = 2
PS_S_BUFS = 2

# Variable-width tiles: small head/tail shrink pipeline fill/drain.
TILE_WIDTHS = [1024, 1024, 2048] + [4096] * 14 + [2048, 1024, 1024]
assert sum(TILE_WIDTHS) == F
TILE_OFFS = [sum(TILE_WIDTHS[:i]) for i in range(len(TILE_WIDTHS))]
NT = len(TILE_WIDTHS)
ADD_N = 1024               # add sub-chunk
OUT_N = 1024               # relu + out sub-chunk
EXP_N = 4096               # ACT exp sub-chunk
SKEWS = (0, 1, 2, 3, 4)
# prelu chunks on DVE custom op (of 64 PSUM chunks); rest on ACT Prelu
# (v6: hill-climbed against TimelineSim; base pattern = first four + every
# 8th, minus 56)
PRELU_DVE_SET = frozenset({0, 1, 2, 3, 8, 16, 24, 32, 40, 48})
# per-1024-chunk engine routes for add and relu ("dve" | "pool"); index =
# (TILE_OFFS + ko) // 1024, 64 chunks total. Base pattern: even chunks
# below 62 on Pool (v5 same-chunk pairing); hill-climbed deltas: add1 ->
# pool, add52 -> dve, relu62 -> pool.
ADD_ROUTE = {c: ("pool" if (c % 2 == 0 and c < 62) else "dve") for c in range(64)}
ADD_ROUTE[1] = "pool"
ADD_ROUTE[52] = "dve"
RELU_ROUTE = {c: ("pool" if (c % 2 == 0 and c <= 62) else "dve") for c in range(64)}
RELU_ROUTE[46] = "dve"
EMIT_ORDER = (0, 1, 2, 3, 4)
OUT_DMA = "scalar"          # engine queue issuing output DMAs
# Scheduler steering: when set, each (stage, tile) emission is gated at
# step*GATE_STEP_MS in the scheduling sim (tile_wait_until), forcing a
# strict round-robin pipeline order. Does NOT affect TimelineSim cost.
GATE_STEP_MS = None
GATE_SUB = (0.0, 0.2, 0.4, 0.6, 0.8)  # per-stage sub-offsets within a step

_CACHE = {}

RECIP_MUL_NAME = "RECIP_MUL_NN11888"
# Chebyshev-minimax seed pair (see RECIP_APPROX_FAST_CONSTS); one NR pass
# instead of two leaves a uop stage for the fused multiply. ~0.4% rel err.
RM_C0 = -0.23549792
RM_C1 = 2.0017324


def _recip_mul_op():
    """Fused DVE op: out = in1 * approx_recip(in0), one NR pass.
    y0 = bitcast(~in0)*C0;  out = Src1 * (y0 * (C1 - Src0*y0)).
    Registered at runtime via the dve_ops extension registry."""
    import numpy as np_
    import concourse.dve_ops as dve_ops
    from concourse.dve_spec import Spec, Src0, Src1, C0, C1, Bin, AluOp, lower, _has_src1
    from concourse.dve_uop import DveOpSpec

    for op in dve_ops.OPS:
        if op.name == RECIP_MUL_NAME:
            return op

    def ref(in0, in1, s0, s1, imm2):
        not_x = (~in0.view(np_.int32)).view(np_.float32)
        y0 = not_x * s0
        return in1 * (y0 * (s1 - in0 * y0))

    _not_x = Bin(AluOp.BITWISE_NOT, Src0, Src0)
    _y0 = _not_x * C0
    spec = Spec(body=Src1 * (_y0 * (C1 - Src0 * _y0)), reference=ref)
    return _register_dve_op(RECIP_MUL_NAME, spec)


PRELU_NAME = "PRELU_PSUM_NN11888"


def _prelu_op():
    """Single-source DVE prelu: out = maxx(C0*Src0, Src0) (slope C0 < 1).
    One PSUM read -> passes the one-PSUM-operand verifier rule."""
    import concourse.dve_ops as dve_ops
    from concourse.dve_spec import Spec, Src0, C0, maxx

    for op in dve_ops.OPS:
        if op.name == PRELU_NAME:
            return op
    spec = Spec(
        body=maxx(Src0 * C0, Src0),
        reference=lambda in0, in1, s0, s1, imm2: __import__("numpy").maximum(
            in0 * s0, in0
        ),
    )
    return _register_dve_op(PRELU_NAME, spec)


def _register_dve_op(name, spec):
    import concourse.dve_ops as dve_ops
    from concourse.dve_spec import lower, _has_src1
    from concourse.dve_uop import DveOpSpec

    op = dve_ops.DveOp(name, spec, subdim=False, uops_sha={})
    row = max(dve_ops._SUB_OPCODE_FOR_NAME.values()) + 1
    assert row < 0x20
    dve_ops.OPS.append(op)
    dve_ops._SUB_OPCODE_FOR_NAME[name] = row
    dve_ops.CUSTOM_DVE_SPECS[name] = spec
    for ver in ("v3", "v4"):
        dve_ops._COMPILE_CACHE[(name, ver)] = DveOpSpec(
            name=name,
            opcode=row,
            uops=lower(spec, ver=ver),
            rd1_en=_has_src1(spec),
        )
    return op


def _build_nc():
    import concourse.bacc as bacc
    import concourse.mybir as mybir
    import concourse.tile as tile

    f32 = mybir.dt.float32
    f16 = mybir.dt.float16
    AF = mybir.ActivationFunctionType
    OP = mybir.AluOpType

    nc = bacc.Bacc(
        "TRN2",
        target_bir_lowering=False,
        debug=False,
        enable_asserts=False,
    )

    xs_d = nc.dram_tensor("xs", [P, F], f16, kind="ExternalInput").ap()
    wblk_d = nc.dram_tensor("wblk", [P, P], f16, kind="ExternalInput").ap()
    ones_d = nc.dram_tensor("onesblk", [P, P], f16, kind="ExternalInput").ap()
    out_d = nc.dram_tensor("out", [P, F], f16, kind="ExternalOutput").ap()

    OUT_DMA_ENG = getattr(nc, OUT_DMA)

    with tile.TileContext(nc) as tc:
        with (
            tc.tile_pool(name="const", bufs=1) as const,
            tc.tile_pool(name="io", bufs=3) as io,
            tc.tile_pool(name="mid", bufs=3) as mid,
            tc.tile_pool(name="ps_v", bufs=PS_V_BUFS, space="PSUM") as ps_v,
            tc.tile_pool(name="ps_s", bufs=PS_S_BUFS, space="PSUM") as ps_s,
        ):
            w_mix = const.tile([P, P], f16)
            nc.scalar.dma_start(out=w_mix[:], in_=wblk_d[:])
            onesblk = const.tile([P, P], f16)
            nc.scalar.dma_start(out=onesblk[:], in_=ones_d[:])

            st = {}  # per-tile live state

            def stage_load(ti):
                off, w = TILE_OFFS[ti], TILE_WIDTHS[ti]
                xs_t = io.tile(
                    [P, w], f16, bufs=(X_BUFS if w == 4096 else 2), name=f"xs_{ti}", tag=f"xs_{w}"
                )
                nc.sync.dma_start(out=xs_t[:], in_=xs_d[:, off : off + w])
                st[ti] = {"xs": xs_t}

            def stage_exp(ti):
                w = TILE_WIDTHS[ti]
                e_t = mid.tile(
                    [P, w], f16, name=f"e_{ti}", tag=f"e_{w}", bufs=(E_BUFS if w == 4096 else 2)
                )
                # e = exp(10 * xs) = exp(x)
                for ko in range(0, w, EXP_N):
                    sl = slice(ko, min(ko + EXP_N, w))
                    nc.scalar.activation(
                        e_t[:, sl], st[ti]["xs"][:, sl], AF.Exp, scale=10.0
                    )
                st[ti]["e"] = e_t

            recip_mul = _recip_mul_op()
            prelu_op = _prelu_op()

            def stage_mm(ti):
                """Per 1024-chunk: V'/SB matmuls, ACT prelu, DVE fused
                t = aa * recip(SB)."""
                w = TILE_WIDTHS[ti]
                e_t = st[ti]["e"]
                aa_t = mid.tile(
                    [P, w], f16, name=f"aa_{ti}", tag=f"aa_{w}", bufs=(AA_BUFS if w == 4096 else 2)
                )
                t_t = mid.tile(
                    [P, w], f16, name=f"t_{ti}", tag=f"t_{w}", bufs=(T_BUFS if w == 4096 else 2)
                )
                for ks in range(0, w, PS_S_N):
                    sw = min(PS_S_N, w - ks)
                    s_c = ps_s.tile([P, PS_S_N], f32, tag="s_c")
                    for kp in range(ks, ks + sw, PS_V_N):
                        cw = min(PS_V_N, w - kp)
                        v_c = ps_v.tile([P, PS_V_N], f32, tag="v_c")
                        for k in range(kp, kp + cw, MM_N):
                            nc.tensor.matmul(
                                v_c[:, k - kp : k - kp + MM_N],
                                w_mix[:],
                                e_t[:, k : k + MM_N],
                                start=True,
                                stop=True,
                            )
                        c = (TILE_OFFS[ti] + kp) // 1024
                        if c in PRELU_DVE_SET:
                            nc.vector._custom_dve(
                                prelu_op,
                                out=aa_t[:, kp : kp + cw],
                                in0=v_c[:, :cw],
                                s0=RRELU_SLOPE,
                            )
                        else:
                            nc.scalar.activation(
                                aa_t[:, kp : kp + cw], v_c[:, :cw], AF.Prelu,
                                bias=0.0, scale=1.0, alpha=RRELU_SLOPE,
                            )
                    for k in range(ks, ks + sw, MM_N):
                        nc.tensor.matmul(
                            s_c[:, k - ks : k - ks + MM_N],
                            onesblk[:],
                            e_t[:, k : k + MM_N],
                            start=True,
                            stop=True,
                        )
                    nc.vector._custom_dve(
                        recip_mul,
                        out=t_t[:, ks : ks + sw],
                        in0=s_c[:, :sw],
                        in1=aa_t[:, ks : ks + sw],
                        s0=RM_C0, s1=RM_C1,
                    )
                st[ti]["t"] = t_t

            def stage_add(ti):
                z_t = io.tile(
                    [P, TILE_WIDTHS[ti]], f16, bufs=(Z_BUFS if TILE_WIDTHS[ti] == 4096 else 2),
                    name=f"z_{ti}", tag=f"z_{TILE_WIDTHS[ti]}"
                )
                xs_t = st[ti].pop("xs")
                t_t = st[ti].pop("t")
                w = TILE_WIDTHS[ti]
                for ko in range(0, w, ADD_N):
                    sl = slice(ko, min(ko + ADD_N, w))
                    c1 = (TILE_OFFS[ti] + ko) // 1024
                    eng = nc.vector if ADD_ROUTE[c1] == "dve" else nc.gpsimd
                    eng.tensor_tensor(
                        z_t[:, sl], xs_t[:, sl], t_t[:, sl], OP.add
                    )
                st[ti]["z"] = z_t

            def stage_relu_out(ti):
                z_t = st[ti]["z"]
                off, w = TILE_OFFS[ti], TILE_WIDTHS[ti]
                for ko in range(0, w, OUT_N):
                    we = min(ko + OUT_N, w)
                    sl = slice(ko, we)
                    c1 = (off + ko) // 1024
                    eng = nc.vector if RELU_ROUTE[c1] == "dve" else nc.gpsimd
                    eng.tensor_scalar(
                        out=z_t[:, sl], in0=z_t[:, sl],
                        scalar1=0.0, scalar2=None, op0=OP.max,
                    )
                    OUT_DMA_ENG.dma_start(
                        out=out_d[:, off + ko : off + we], in_=z_t[:, sl]
                    )
                del st[ti]

            stages = [stage_load, stage_exp, stage_mm,
                      stage_add, stage_relu_out]
            offs = SKEWS
            maxoff = max(offs)
            import contextlib
            for step in range(NT + maxoff):
                for si in EMIT_ORDER:
                    ti = step - offs[si]
                    if 0 <= ti < NT:
                        if GATE_STEP_MS is not None:
                            gate = (step + GATE_SUB[si]) * GATE_STEP_MS
                            cm = tc.tile_wait_until(gate)
                        else:
                            cm = contextlib.nullcontext()
                        with cm:
                            stages[si](ti)

    nc.compile()
    return nc


def _get_nc():
    if "nc" not in _CACHE:
        _CACHE["nc"] = _build_nc()
    return _CACHE["nc"]


def _make_in_maps(x, mix, bias):
    x = np.asarray(x, dtype=np.float32)
    mix = np.asarray(mix, dtype=np.float32)
    bias = np.asarray(bias, dtype=np.float32)

    xs = np.ascontiguousarray(
        (0.1 * x).reshape(N_CORES, P, F).astype(np.float16)
    )

    # lhsT layout: V'[(b,d),n] = sum_{(b',c)} wblk[(b',c),(b,d)] * e[(b',c),n]
    # wblk[(b,c),(b,d)] = mix[d,c] + bias[d]  (bias folded: sums to bias*S)
    blk = (mix.T + bias[None, :]).astype(np.float16)
    wblk = np.zeros((P, P), np.float16)
    wblk[0:C, 0:C] = blk
    wblk[C : 2 * C, C : 2 * C] = blk

    onesblk = np.zeros((P, P), np.float16)
    onesblk[0:C, 0:C] = 1.0
    onesblk[C : 2 * C, C : 2 * C] = 1.0

    return [
        {"xs": xs[c], "wblk": wblk, "onesblk": onesblk}
        for c in range(N_CORES)
    ]


def run(inputs, trace=False):
    from concourse.bass_utils import run_bass_kernel_spmd

    nc = _get_nc()
    in_maps = _make_in_maps(inputs["x"], inputs["mix"], inputs["bias"])
    res = run_bass_kernel_spmd(nc, in_maps, list(range(N_CORES)), trace=trace)
    out = np.stack([res.results[c]["out"] for c in range(N_CORES)])
    return out.reshape(B, C, H, W).astype(np.float32), res


def kernel(x, mix, bias):
    out, _ = run({"x": x, "mix": mix, "bias": bias})
    return out



# revision 26
# speedup vs baseline: 1.0042x; 1.0042x over previous
"""Trainium2 Bass kernel for: softmax2d(channel) -> channel mix -> bias ->
RReLU(0.2 eval) -> relu(mixed + 0.1*x).

Full-input contract: kernel(**inputs) takes the complete tensors and returns
the complete output. Internally shards batch B=16 across 8 NeuronCores
(2 batches/core). Per-core layout: [128 partitions = 2 batches x 64 channels,
65536 free = H*W].

v5: fp16 I/O (DMA 187->93 us/core) + fused recip-multiply custom DVE op.

Host uploads xs = f16(0.1*x); ACT exp uses scale=10 so e = exp(x).
With W'[(b,c),(b,d)] = mix[d,c] + bias[d] (bias folded via sum_c e = S):
  V' = W' @ e = S*(mix@softmax + bias),  SB = blockones @ e = S (bcast)
  aa = prelu(V')   ACT Prelu (psum->f16; shares exp's act table) with a few
                   chunks on a custom DVE op maxx(0.2*v, v) for balance
  t  = aa * 1/SB   ONE custom DVE op (bit-trick seed + 1 Newton + multiply,
                   ~0.4% rel err) - replaces recip + full multiply pass
  z  = relu(xs+t)  tt-add f16 (2x) + ts-max f16 (4x); even chunks take both
                   steps on Pool (same-chunk pairing schedules best)

ISA constraints found the hard way: no divide op on any engine; at most one
PSUM operand per DVE/Pool instruction (even the same AP twice); Pool reads
SBUF only and only tt/ts classes (TensorScalarPtr scan variant is rejected
by walrus codegen on Pool); custom/stt DVE ops get no 2x/4x modes; matmul
outputs f32 PSUM only; DMA cannot read PSUM (so S can never be compacted
out of PSUM cheaply - engines charge per free-column regardless of
partition count); AF.Reciprocal blocked, exp/reciprocal tables thrash.
Small head/tail tiles trim pipeline fill and drain.

v6: per-chunk engine routing tables (ADD_ROUTE/RELU_ROUTE/PRELU_DVE_SET)
hill-climbed against TimelineSim. The v5 even/odd same-chunk pairing is a
sharp scheduler local optimum: the tile list-scheduler costs Pool with v1
(no gpsimd efficiency penalty, 2.4x optimistic for tt-add), so any large
rebalance (wider Pool chunks, scan-adds, all-relu-on-DVE, gated manual
schedules, 2048 PSUM chunks) lowers busy but adds 5-70us of queue-convoy
stalls. Only single-chunk boundary flips survived: add1->pool,
add52->dve, relu62->pool, relu46->dve, prelu56->ACT. Engine busy
(TimelineSim): DVE 118.6, ACT 115.6, Pool 113.4, DMA 93.6, PE 56.6 ->
exec 133.46 us (vs 134.45 v5, 196.6 original).
"""

import numpy as np

B, C, H, W = 16, 64, 256, 256
N_CORES = 8
BPC = B // N_CORES          # batches per core
P = BPC * C                 # 128 partitions
F = H * W                   # 65536 free columns per core
TILE_N = 4096               # SBUF tile width
PS_V_N = 1024               # V' PSUM chunk width (prelu granularity)
PS_S_N = 1024               # S PSUM chunk width (recip_mul granularity)
MM_N = 512                 # single matmul free dim (1 PSUM bank)
RRELU_SLOPE = 0.2
X_BUFS = 5
E_BUFS = 3
AA_BUFS = 3
T_BUFS = 3
Z_BUFS = 4
PS_V_BUFS = 2
PS_S_BUFS = 2

# Variable-width tiles: small head/tail shrink pipeline fill/drain.
TILE_WIDTHS = [1024, 1024, 2048] + [4096] * 14 + [2048, 1024, 1024]
assert sum(TILE_WIDTHS) == F
TILE_OFFS = [sum(TILE_WIDTHS[:i]) for i in range(len(TILE_WIDTHS))]
NT = len(TILE_WIDTHS)
ADD_N = 1024               # add sub-chunk
OUT_N = 1024               # relu + out sub-chunk
EXP_N = 4096               # ACT exp sub-chunk
SKEWS = (0, 1, 2, 3, 4)
# prelu chunks on DVE custom op (of 64 PSUM chunks); rest on ACT Prelu
# (v6: hill-climbed against TimelineSim; base pattern = first four + every
# 8th, minus 56)
PRELU_DVE_SET = frozenset({0, 1, 2, 3, 8, 16, 24, 32, 40, 48})
# per-1024-chunk engine routes for add and relu ("dve" | "pool"); index =
# (TILE_OFFS + ko) // 1024, 64 chunks total. Base pattern: even chunks
# below 62 on Pool (v5 same-chunk pairing); hill-climbed deltas: add1 ->
# pool, add52 -> dve, relu62 -> pool.
ADD_ROUTE = {c: ("pool" if (c % 2 == 0 and c < 62) else "dve") for c in range(64)}
ADD_ROUTE[1] = "pool"
ADD_ROUTE[52] = "dve"
RELU_ROUTE = {c: ("pool" if (c % 2 == 0 and c <= 62) else "dve") for c in range(64)}
RELU_ROUTE[46] = "dve"
EMIT_ORDER = (0, 1, 2, 3, 4)
OUT_DMA = "scalar"          # engine queue issuing output DMAs
# Scheduler steering: when set, each (stage, tile) emission is gated at
# step*GATE_STEP_MS in the scheduling sim (tile_wait_until), forcing a
# strict round-robin pipeline order. Does NOT affect TimelineSim cost.
GATE_STEP_MS = None
GATE_SUB = (0.0, 0.2, 0.4, 0.6, 0.8)  # per-stage sub-offsets within a step
S_FIRST = False            # emit S-matmuls before V-matmuls within a chunk
# output DMA queue per 1024-chunk ("scalar" | "sync"); head/tail chunks go
# on the otherwise-quiet SP queue so their DMAs don't stall ACT's SEQ
# during fill and drain (hill-climbed: -0.4us)
OUT_Q_ROUTE = {c: "scalar" for c in range(64)}
for _c in (1, 2, 61, 62, 63):
    OUT_Q_ROUTE[_c] = "sync"

_CACHE = {}

RECIP_MUL_NAME = "RECIP_MUL_NN11888"
# Chebyshev-minimax seed pair (see RECIP_APPROX_FAST_CONSTS); one NR pass
# instead of two leaves a uop stage for the fused multiply. ~0.4% rel err.
RM_C0 = -0.23549792
RM_C1 = 2.0017324


def _recip_mul_op():
    """Fused DVE op: out = in1 * approx_recip(in0), one NR pass.
    y0 = bitcast(~in0)*C0;  out = Src1 * (y0 * (C1 - Src0*y0)).
    Registered at runtime via the dve_ops extension registry."""
    import numpy as np_
    import concourse.dve_ops as dve_ops
    from concourse.dve_spec import Spec, Src0, Src1, C0, C1, Bin, AluOp, lower, _has_src1
    from concourse.dve_uop import DveOpSpec

    for op in dve_ops.OPS:
        if op.name == RECIP_MUL_NAME:
            return op

    def ref(in0, in1, s0, s1, imm2):
        not_x = (~in0.view(np_.int32)).view(np_.float32)
        y0 = not_x * s0
        return in1 * (y0 * (s1 - in0 * y0))

    _not_x = Bin(AluOp.BITWISE_NOT, Src0, Src0)
    _y0 = _not_x * C0
    spec = Spec(body=Src1 * (_y0 * (C1 - Src0 * _y0)), reference=ref)
    return _register_dve_op(RECIP_MUL_NAME, spec)


PRELU_NAME = "PRELU_PSUM_NN11888"


def _prelu_op():
    """Single-source DVE prelu: out = maxx(C0*Src0, Src0) (slope C0 < 1).
    One PSUM read -> passes the one-PSUM-operand verifier rule."""
    import concourse.dve_ops as dve_ops
    from concourse.dve_spec import Spec, Src0, C0, maxx

    for op in dve_ops.OPS:
        if op.name == PRELU_NAME:
            return op
    spec = Spec(
        body=maxx(Src0 * C0, Src0),
        reference=lambda in0, in1, s0, s1, imm2: __import__("numpy").maximum(
            in0 * s0, in0
        ),
    )
    return _register_dve_op(PRELU_NAME, spec)


def _register_dve_op(name, spec):
    import concourse.dve_ops as dve_ops
    from concourse.dve_spec import lower, _has_src1
    from concourse.dve_uop import DveOpSpec

    op = dve_ops.DveOp(name, spec, subdim=False, uops_sha={})
    row = max(dve_ops._SUB_OPCODE_FOR_NAME.values()) + 1
    assert row < 0x20
    dve_ops.OPS.append(op)
    dve_ops._SUB_OPCODE_FOR_NAME[name] = row
    dve_ops.CUSTOM_DVE_SPECS[name] = spec
    for ver in ("v3", "v4"):
        dve_ops._COMPILE_CACHE[(name, ver)] = DveOpSpec(
            name=name,
            opcode=row,
            uops=lower(spec, ver=ver),
            rd1_en=_has_src1(spec),
        )
    return op


def _build_nc():
    import concourse.bacc as bacc
    import concourse.mybir as mybir
    import concourse.tile as tile

    f32 = mybir.dt.float32
    f16 = mybir.dt.float16
    AF = mybir.ActivationFunctionType
    OP = mybir.AluOpType

    nc = bacc.Bacc(
        "TRN2",
        target_bir_lowering=False,
        debug=False,
        enable_asserts=False,
    )

    xs_d = nc.dram_tensor("xs", [P, F], f16, kind="ExternalInput").ap()
    wblk_d = nc.dram_tensor("wblk", [P, P], f16, kind="ExternalInput").ap()
    ones_d = nc.dram_tensor("onesblk", [P, P], f16, kind="ExternalInput").ap()
    out_d = nc.dram_tensor("out", [P, F], f16, kind="ExternalOutput").ap()

    OUT_DMA_ENG = getattr(nc, OUT_DMA)

    with tile.TileContext(nc) as tc:
        with (
            tc.tile_pool(name="const", bufs=1) as const,
            tc.tile_pool(name="io", bufs=3) as io,
            tc.tile_pool(name="mid", bufs=3) as mid,
            tc.tile_pool(name="ps_v", bufs=PS_V_BUFS, space="PSUM") as ps_v,
            tc.tile_pool(name="ps_s", bufs=PS_S_BUFS, space="PSUM") as ps_s,
        ):
            w_mix = const.tile([P, P], f16)
            nc.scalar.dma_start(out=w_mix[:], in_=wblk_d[:])
            onesblk = const.tile([P, P], f16)
            nc.scalar.dma_start(out=onesblk[:], in_=ones_d[:])

            st = {}  # per-tile live state

            def stage_load(ti):
                off, w = TILE_OFFS[ti], TILE_WIDTHS[ti]
                xs_t = io.tile(
                    [P, w], f16, bufs=(X_BUFS if w == 4096 else 2), name=f"xs_{ti}", tag=f"xs_{w}"
                )
                nc.sync.dma_start(out=xs_t[:], in_=xs_d[:, off : off + w])
                st[ti] = {"xs": xs_t}

            def stage_exp(ti):
                w = TILE_WIDTHS[ti]
                e_t = mid.tile(
                    [P, w], f16, name=f"e_{ti}", tag=f"e_{w}", bufs=(E_BUFS if w == 4096 else 2)
                )
                # e = exp(10 * xs) = exp(x)
                for ko in range(0, w, EXP_N):
                    sl = slice(ko, min(ko + EXP_N, w))
                    nc.scalar.activation(
                        e_t[:, sl], st[ti]["xs"][:, sl], AF.Exp, scale=10.0
                    )
                st[ti]["e"] = e_t

            recip_mul = _recip_mul_op()
            prelu_op = _prelu_op()

            def stage_mm(ti):
                """Per 1024-chunk: V'/SB matmuls, ACT prelu, DVE fused
                t = aa * recip(SB)."""
                w = TILE_WIDTHS[ti]
                e_t = st[ti]["e"]
                aa_t = mid.tile(
                    [P, w], f16, name=f"aa_{ti}", tag=f"aa_{w}", bufs=(AA_BUFS if w == 4096 else 2)
                )
                t_t = mid.tile(
                    [P, w], f16, name=f"t_{ti}", tag=f"t_{w}", bufs=(T_BUFS if w == 4096 else 2)
                )
                def emit_s_mms(s_c, ks, sw):
                    for k in range(ks, ks + sw, MM_N):
                        nc.tensor.matmul(
                            s_c[:, k - ks : k - ks + MM_N],
                            onesblk[:],
                            e_t[:, k : k + MM_N],
                            start=True,
                            stop=True,
                        )

                for ks in range(0, w, PS_S_N):
                    sw = min(PS_S_N, w - ks)
                    s_c = ps_s.tile([P, PS_S_N], f32, tag="s_c")
                    if S_FIRST:
                        emit_s_mms(s_c, ks, sw)
                    for kp in range(ks, ks + sw, PS_V_N):
                        cw = min(PS_V_N, w - kp)
                        v_c = ps_v.tile([P, PS_V_N], f32, tag="v_c")
                        for k in range(kp, kp + cw, MM_N):
                            nc.tensor.matmul(
                                v_c[:, k - kp : k - kp + MM_N],
                                w_mix[:],
                                e_t[:, k : k + MM_N],
                                start=True,
                                stop=True,
                            )
                        c = (TILE_OFFS[ti] + kp) // 1024
                        if c in PRELU_DVE_SET:
                            nc.vector._custom_dve(
                                prelu_op,
                                out=aa_t[:, kp : kp + cw],
                                in0=v_c[:, :cw],
                                s0=RRELU_SLOPE,
                            )
                        else:
                            nc.scalar.activation(
                                aa_t[:, kp : kp + cw], v_c[:, :cw], AF.Prelu,
                                bias=0.0, scale=1.0, alpha=RRELU_SLOPE,
                            )
                    if not S_FIRST:
                        emit_s_mms(s_c, ks, sw)
                    nc.vector._custom_dve(
                        recip_mul,
                        out=t_t[:, ks : ks + sw],
                        in0=s_c[:, :sw],
                        in1=aa_t[:, ks : ks + sw],
                        s0=RM_C0, s1=RM_C1,
                    )
                st[ti]["t"] = t_t

            def stage_add(ti):
                z_t = io.tile(
                    [P, TILE_WIDTHS[ti]], f16, bufs=(Z_BUFS if TILE_WIDTHS[ti] == 4096 else 2),
                    name=f"z_{ti}", tag=f"z_{TILE_WIDTHS[ti]}"
                )
                xs_t = st[ti].pop("xs")
                t_t = st[ti].pop("t")
                w = TILE_WIDTHS[ti]
                for ko in range(0, w, ADD_N):
                    sl = slice(ko, min(ko + ADD_N, w))
                    c1 = (TILE_OFFS[ti] + ko) // 1024
                    eng = nc.vector if ADD_ROUTE[c1] == "dve" else nc.gpsimd
                    eng.tensor_tensor(
                        z_t[:, sl], xs_t[:, sl], t_t[:, sl], OP.add
                    )
                st[ti]["z"] = z_t

            def stage_relu_out(ti):
                z_t = st[ti]["z"]
                off, w = TILE_OFFS[ti], TILE_WIDTHS[ti]
                for ko in range(0, w, OUT_N):
                    we = min(ko + OUT_N, w)
                    sl = slice(ko, we)
                    c1 = (off + ko) // 1024
                    eng = nc.vector if RELU_ROUTE[c1] == "dve" else nc.gpsimd
                    eng.tensor_scalar(
                        out=z_t[:, sl], in0=z_t[:, sl],
                        scalar1=0.0, scalar2=None, op0=OP.max,
                    )
                    q = getattr(nc, OUT_Q_ROUTE.get(c1, OUT_DMA))
                    q.dma_start(
                        out=out_d[:, off + ko : off + we], in_=z_t[:, sl]
                    )
                del st[ti]

            stages = [stage_load, stage_exp, stage_mm,
                      stage_add, stage_relu_out]
            offs = SKEWS
            maxoff = max(offs)
            import contextlib
            for step in range(NT + maxoff):
                for si in EMIT_ORDER:
                    ti = step - offs[si]
                    if 0 <= ti < NT:
                        if GATE_STEP_MS is not None:
                            gate = (step + GATE_SUB[si]) * GATE_STEP_MS
                            cm = tc.tile_wait_until(gate)
                        else:
                            cm = contextlib.nullcontext()
                        with cm:
                            stages[si](ti)

    nc.compile()
    return nc


def _get_nc():
    if "nc" not in _CACHE:
        _CACHE["nc"] = _build_nc()
    return _CACHE["nc"]


def _make_in_maps(x, mix, bias):
    x = np.asarray(x, dtype=np.float32)
    mix = np.asarray(mix, dtype=np.float32)
    bias = np.asarray(bias, dtype=np.float32)

    xs = np.ascontiguousarray(
        (0.1 * x).reshape(N_CORES, P, F).astype(np.float16)
    )

    # lhsT layout: V'[(b,d),n] = sum_{(b',c)} wblk[(b',c),(b,d)] * e[(b',c),n]
    # wblk[(b,c),(b,d)] = mix[d,c] + bias[d]  (bias folded: sums to bias*S)
    blk = (mix.T + bias[None, :]).astype(np.float16)
    wblk = np.zeros((P, P), np.float16)
    wblk[0:C, 0:C] = blk
    wblk[C : 2 * C, C : 2 * C] = blk

    onesblk = np.zeros((P, P), np.float16)
    onesblk[0:C, 0:C] = 1.0
    onesblk[C : 2 * C, C : 2 * C] = 1.0

    return [
        {"xs": xs[c], "wblk": wblk, "onesblk": onesblk}
        for c in range(N_CORES)
    ]


def run(inputs, trace=False):
    from concourse.bass_utils import run_bass_kernel_spmd

    nc = _get_nc()
    in_maps = _make_in_maps(inputs["x"], inputs["mix"], inputs["bias"])
    res = run_bass_kernel_spmd(nc, in_maps, list(range(N_CORES)), trace=trace)
    out = np.stack([res.results[c]["out"] for c in range(N_CORES)])
    return out.reshape(B, C, H, W).astype(np.float32), res


def kernel(x, mix, bias):
    out, _ = run({"x": x, "mix": mix, "bias": bias})
    return out

